# revision 17
# baseline (speedup 1.0000x reference)
"""Trainium2 Bass kernel for nn_Binder_MiniGrid (topk_masking).

Algebraic reduction: with q == bq constant, attention scores collapse to a
single linear functional of the LN'd conv features, which folds into two
extra output columns of the conv matmul (s1 = f @ (wk^T bq * ln1_g), mu).
Only the conv itself (451 GFLOP total) runs dense; k/v projections are never
materialized. Top-4 masking + softmax happens on a (1, 1968) score row per
sample (Max8). Pooling runs as a scalar_tensor_tensor accumulation chain on
the vector engine plus 3 tiny partition-sum matmuls per sample (keeping the
PE conv stream uninterrupted), and the final wv projection + LN2 + ReLU run
once per core on (32, 256).

Data parallel over 8 NeuronCores: 32 of the 256 stacked (curr; targ) samples
per core. Conv runs in float32r (TF32-like, 12-bit mantissa) at full PE rate;
measured end-to-end error vs the fp32 reference ~1.9e-4; ~1.14 ms/core.
"""
import sys
sys.path.insert(0, "/opt/trn_rl_repo")
import numpy as np

import concourse.bacc as bacc
import concourse.tile as tile
from concourse import mybir
from concourse.bass_types import AP as BassAP
from concourse.bass_utils import run_bass_kernel_spmd

F32 = mybir.dt.float32
F32R = mybir.dt.float32r
AF = mybir.ActivationFunctionType
ALU = mybir.AluOpType

B, C, HH, WW = 128, 32, 48, 48
E, KF, TOPK, EPS = 256, 8, 4, 1e-5
OH = 41
L = OH * OH            # 1681
NS = 32                # samples per core
N_CORES = 8
NT = 16                # M-tiles per sample: 16 x 123 = 1968 flat 48-wide positions
XW = 2308              # X4 tile width (48*48 data + tap/slice slack)
XWP = XW + 3           # padded host row width (j-shift headroom)
NE = E + 2             # 258: conv channels + s1 + mu columns
NEG = -1.0e9

_CACHE = {}


def _round_f32r(a):
    """Round fp32 to fp32r (round-to-nearest into 12-bit mantissa)."""
    bits = np.ascontiguousarray(a, np.float32).view(np.uint32).astype(np.uint64)
    r = ((bits + 0x800) & ~np.uint64(0xFFF)).astype(np.uint32)
    return r.view(np.float32).reshape(a.shape)


def build_nc(ns=NS, pipelined=True, bufs=4):
    nc = bacc.Bacc()
    x_t = nc.dram_tensor("x", [ns, C, XWP], F32R, kind="ExternalInput")
    warr_ap = nc.dram_tensor("warr", [128, 16 * NE], F32R, kind="ExternalInput").ap()
    brd_ap = nc.dram_tensor("brd", [128, NE], F32, kind="ExternalInput").ap()
    wvg_ap = nc.dram_tensor("wvg", [128, 2 * E], F32, kind="ExternalInput").ap()
    cc_ap = nc.dram_tensor("cc", [2, E], F32, kind="ExternalInput").ap()
    g2b2_ap = nc.dram_tensor("g2b2", [NS, 2 * E], F32, kind="ExternalInput").ap()
    mjunk_ap = nc.dram_tensor("mjunk", [1, 123 * NT], F32, kind="ExternalInput").ap()
    out_ap = nc.dram_tensor("out", [ns, E], F32, kind="ExternalOutput").ap()

    with tile.TileContext(nc) as tc:
        with tc.tile_pool(name="const", bufs=1) as cpool, \
             tc.tile_pool(name="samp", bufs=bufs) as sp, \
             tc.tile_pool(name="small", bufs=3) as smp, \
             tc.tile_pool(name="rows", bufs=2) as rp, \
             tc.tile_pool(name="psum", bufs=bufs, space="PSUM") as pp, \
             tc.tile_pool(name="psum_acc", bufs=1, space="PSUM") as pacc:

            # ---- static loads
            w_sb = cpool.tile([128, 16 * NE], F32R)
            nc.sync.dma_start(w_sb[:], warr_ap[:])
            brd = cpool.tile([128, NE], F32)
            nc.sync.dma_start(brd[:], brd_ap[:])
            wvg = cpool.tile([128, 2 * E], F32)
            nc.sync.dma_start(wvg[:], wvg_ap[:])
            cc = cpool.tile([2, E], F32)
            nc.sync.dma_start(cc[:], cc_ap[:])
            g2b2 = cpool.tile([NS, 2 * E], F32)
            nc.sync.dma_start(g2b2[:], g2b2_ap[:])
            textra = cpool.tile([2, NS], F32)
            nc.vector.memset(textra[0:2, :], 1.0)  # row 0 overwritten by T DMA below
            c_eeps = cpool.tile([128, 1], F32)
            nc.vector.memset(c_eeps[:], float(E) * EPS)
            c_eps = cpool.tile([NS, 1], F32)
            nc.vector.memset(c_eps[:], EPS)
            ones_col = cpool.tile([128, 1], F32)
            nc.vector.memset(ones_col[:], 1.0)
            mjunk = cpool.tile([1, 123 * NT], F32)
            nc.sync.dma_start(mjunk[:], mjunk_ap[:])

            pooled_all = pacc.tile([128, 3 * ns], F32)
            nc.vector.memset(pooled_all[:], 0.0)

            prev = None
            for s in range(ns):
                # ---- X4: partition (c*4+j) = x[s, c, j:j+XW]  (single DMA)
                x4 = sp.tile([128, XW], F32R)
                src = BassAP(x_t, s * C * XWP, [[XWP, C], [1, 4], [1, XW]])
                nc.sync.dma_start(x4[:, :], src)

                f_s = sp.tile([128, NT * NE], F32)
                ss = smp.tile([128, NT], F32)   # sum of squares per position

                for t in range(NT):
                    pt = pp.tile([128, NE], F32)
                    for ki in range(16):
                        kh, jb = ki // 2, ki % 2
                        off = kh * 48 + jb * 4 + 123 * t
                        lhsT = x4[:, off:off + 123]
                        rhs = w_sb[:, ki * NE:(ki + 1) * NE]
                        nc.tensor.matmul(pt[:123, :], lhsT, rhs,
                                         start=(ki == 0), stop=(ki == 15),
                                         skip_group_check=True)
                    ft = f_s[:, t * NE:(t + 1) * NE]
                    nc.vector.tensor_add(ft[:123, :], pt[:123, :], brd[:123, :])
                    sq = smp.tile([128, E], F32, tag="sq")
                    nc.scalar.activation(sq[:123, :], ft[:123, 0:E], AF.Square,
                                         accum_out=ss[:123, t:t + 1])

                # ---- scores on (123, NT)
                fv = f_s[:, :].rearrange("p (t e) -> p t e", t=NT)
                mu_v = fv[:123, :, E + 1]
                s1_v = fv[:123, :, E]
                sc1 = smp.tile([128, NT], F32, tag="sc1")
                nc.vector.tensor_mul(sc1[:123, :], mu_v, mu_v)
                nc.vector.scalar_tensor_tensor(sc1[:123, :], sc1[:123, :],
                                               -float(E), ss[:123, :],
                                               op0=ALU.mult, op1=ALU.add)
                sig = smp.tile([128, NT], F32, tag="sig")
                nc.scalar.activation(sig[:123, :], sc1[:123, :], AF.Sqrt,
                                     bias=c_eeps[:123, 0:1])
                inv2 = smp.tile([128, NT], F32, tag="inv2")
                nc.vector.reciprocal(inv2[:123, :], sig[:123, :])
                # host folded "- S*mu" into the s1 column: score = s1_v * inv2
                scs = smp.tile([128, NT], F32, tag="scs")
                nc.vector.tensor_mul(scs[:123, :], s1_v, inv2[:123, :])

                # ---- row / top-k (row order: flat p = oh*48 + ow; ow>=41 junk)
                row = rp.tile([1, 123 * NT], F32, tag="row")
                nc.sync.dma_start(row[0:1, :], scs[:123, :])
                # additive -1e9 at wrap positions (ow >= 41), in (p*NT + t) order
                nc.vector.tensor_add(row[:], row[:], mjunk[:])
                m8 = smp.tile([1, 8], F32, tag="m8")
                nc.vector.max(out=m8[:], in_=row[:])
                nvmax = smp.tile([1, 1], F32, tag="nvmax")
                nc.vector.tensor_scalar(nvmax[:], m8[:, 0:1], -1.0, None,
                                        op0=ALU.mult)
                e4 = smp.tile([1, 4], F32, tag="e4")
                sumw = smp.tile([1, 1], F32, tag="sumw")
                nc.scalar.activation(e4[:], m8[:, 0:4], AF.Exp,
                                     bias=nvmax[0:1, 0:1], accum_out=sumw[:])
                lnw = smp.tile([1, 1], F32, tag="lnw")
                nc.scalar.activation(lnw[:], sumw[:], AF.Ln)
                nvl = smp.tile([1, 1], F32, tag="nvl")
                nc.vector.tensor_sub(nvl[:], nvmax[:], lnw[:])
                erow = rp.tile([1, 123 * NT], F32, tag="erow")
                # erow = exp(row - vmax - ln(sumw)) * (row >= kth)
                nc.scalar.activation(erow[:], row[:], AF.Exp,
                                     bias=nvl[0:1, 0:1])
                nc.vector.scalar_tensor_tensor(erow[:], row[:], m8[0:1, 3:4],
                                               erow[:], op0=ALU.is_ge,
                                               op1=ALU.mult)

                # ---- back to (123, NT), * inv2
                w_s = smp.tile([128, NT], F32, tag="w_s")
                nc.sync.dma_start(w_s[:123, :], erow[0:1, :])
                nc.vector.tensor_mul(w_s[:123, :], w_s[:123, :], inv2[:123, :])

                # ---- pooled: DVE weighted accumulation over tiles, then 3
                # tiny PE matmuls (vs ones) for the partition reduction.
                # Software-pipelined one sample back so the score->DMA->softmax
                # chain hides under the next conv.
                def emit_pooled(si, fsi, wsi, first, last):
                    acc = smp.tile([128, NE], F32, tag="acc")
                    nc.vector.tensor_scalar(acc[:123, :], fsi[:123, 0:NE],
                                            wsi[:123, 0:1], None, op0=ALU.mult)
                    for t in range(1, NT):
                        nc.vector.scalar_tensor_tensor(
                            acc[:123, :], fsi[:123, t * NE:(t + 1) * NE],
                            wsi[:123, t:t + 1], acc[:123, :],
                            op0=ALU.mult, op1=ALU.add)
                    for ci, (c0, cm) in enumerate(((0, 128), (128, 128), (256, 2))):
                        nc.tensor.matmul(
                            pooled_all[0:cm, 3 * si + ci:3 * si + ci + 1],
                            acc[:123, c0:c0 + cm], ones_col[:123, 0:1],
                            start=(first and ci == 0), stop=(last and ci == 2),
                            skip_group_check=True)
                if pipelined:
                    if prev is not None:
                        emit_pooled(prev[0], prev[1], prev[2], prev[0] == 0, False)
                    prev = (s, f_s, w_s)
                else:
                    emit_pooled(s, f_s, w_s, s == 0, s == ns - 1)

            if pipelined:
                emit_pooled(prev[0], prev[1], prev[2], prev[0] == 0, True)

            # ---- tail: final projection + LN2 + relu
            psb = cpool.tile([128, 3 * ns], F32)
            nc.scalar.copy(psb[:], pooled_all[:])
            nc.sync.dma_start(textra[0:1, 0:ns], psb[1:2, 2::3])
            outp = pacc.tile([NS, E], F32, tag="outp")
            nc.tensor.matmul(outp[0:ns, :], psb[:, 0::3][:, 0:ns], wvg[:, 0:E],
                             start=True, stop=False, skip_group_check=True)
            nc.tensor.matmul(outp[0:ns, :], psb[:, 1::3][:, 0:ns], wvg[:, E:2 * E],
                             start=False, stop=False, skip_group_check=True)
            nc.tensor.matmul(outp[0:ns, :], textra[:, 0:ns], cc[:],
                             start=False, stop=True, skip_group_check=True)

            srow = cpool.tile([NS, 1], F32)
            nc.vector.reduce_sum(srow[0:ns, :], outp[0:ns, :],
                                 axis=mybir.AxisListType.X)
            nmu = cpool.tile([NS, 1], F32)
            nc.vector.tensor_scalar(nmu[0:ns, :], srow[0:ns, :], -1.0 / E, None,
                                    op0=ALU.mult)
            cent = cpool.tile([NS, E], F32)
            nc.vector.tensor_scalar(cent[0:ns, :], outp[0:ns, :],
                                    nmu[0:ns, 0:1], None, op0=ALU.add)
            sq2 = cpool.tile([NS, E], F32)
            ssq = cpool.tile([NS, 1], F32)
            nc.scalar.activation(sq2[0:ns, :], cent[0:ns, :], AF.Square,
                                 accum_out=ssq[0:ns, :])
            sigf = cpool.tile([NS, 1], F32)
            nc.scalar.activation(sigf[0:ns, :], ssq[0:ns, :], AF.Sqrt,
                                 bias=c_eps[0:ns, 0:1], scale=1.0 / E)
            invf = cpool.tile([NS, 1], F32)
            nc.vector.reciprocal(invf[0:ns, :], sigf[0:ns, :])
            nc.vector.tensor_scalar(cent[0:ns, :], cent[0:ns, :],
                                    invf[0:ns, 0:1], None, op0=ALU.mult)
            nc.vector.tensor_mul(cent[0:ns, :], cent[0:ns, :], g2b2[0:ns, 0:E])
            nc.vector.tensor_add(cent[0:ns, :], cent[0:ns, :], g2b2[0:ns, E:2 * E])
            yout = cpool.tile([NS, E], F32)
            nc.scalar.activation(yout[0:ns, :], cent[0:ns, :], AF.Relu)
            nc.sync.dma_start(out_ap[:], yout[0:ns, :])

    nc.finalize()
    return nc


def host_prep(inputs, ns=NS):
    """Build per-core input maps + the stacked sample array."""
    conv_w = np.asarray(inputs["conv_w"], np.float32)
    conv_b = np.asarray(inputs["conv_b"], np.float32)
    g1 = np.asarray(inputs["ln1_g"], np.float32)
    b1 = np.asarray(inputs["ln1_b"], np.float32)
    wk = np.asarray(inputs["wk"], np.float32)
    bk = np.asarray(inputs["bk"], np.float32)
    bq = np.asarray(inputs["bq"], np.float32)
    wv = np.asarray(inputs["wv"], np.float32)
    bv = np.asarray(inputs["bv"], np.float32)
    g2 = np.asarray(inputs["ln2_g"], np.float32)
    b2 = np.asarray(inputs["ln2_b"], np.float32)

    W2 = conv_w.transpose(1, 2, 3, 0).reshape(C * KF * KF, E)  # [(c,kh,kw), e]
    u = wk.T @ bq
    ug = u * g1
    S = float(ug.sum())
    col_s1 = W2 @ ug - S * (W2 @ (np.ones(E, np.float32) / E))
    # fold "- S*mu" into the s1 column so the device skips that op:
    col_mu = W2 @ (np.ones(E, np.float32) / E)
    W_aug = np.concatenate([W2, col_s1[:, None], col_mu[:, None]], 1)
    b_mu = float(conv_b.mean())
    b_s1 = float(conv_b @ ug) - S * b_mu
    bias_aug = np.concatenate([conv_b, [b_s1], [b_mu]]).astype(np.float32)

    # rearrange W_aug into the 16 (kh, jb) chunks, k = c*4 + j
    warr = np.zeros((128, 16 * NE), np.float32)
    for ki in range(16):
        kh, jb = ki // 2, ki % 2
        for c in range(C):
            for j in range(4):
                warr[c * 4 + j, ki * NE:(ki + 1) * NE] = \
                    W_aug[c * 64 + kh * 8 + jb * 4 + j]
    warr = _round_f32r(warr)

    brd = np.tile(bias_aug[None, :], (128, 1)).astype(np.float32)

    sqE = np.sqrt(np.float32(E))
    wvg_m = (wv * g1[None, :]) * sqE          # (f, e)
    wvgT = wvg_m.T                            # (e, f)
    wvg = np.zeros((128, 2 * E), np.float32)
    wvg[:, 0:E] = wvgT[0:128, :]
    wvg[:, E:2 * E] = wvgT[128:256, :]

    vec_t = wvg_m.sum(axis=1)                 # sqE * sum_e wv*g1
    const_vec = wv @ b1 + bv
    cc = np.stack([-vec_t, const_vec]).astype(np.float32)

    g2b2 = np.zeros((NS, 2 * E), np.float32)
    g2b2[:, 0:E] = g2[None, :]
    g2b2[:, E:2 * E] = b2[None, :]

    mjunk = np.zeros((1, 123 * NT), np.float32)
    for p in range(123):
        for t in range(NT):
            if (123 * t + p) % 48 >= OH:
                mjunk[0, p * NT + t] = NEG

    x_all = np.concatenate([np.asarray(inputs["state_curr"], np.float32),
                            np.asarray(inputs["state_targ"], np.float32)], 0)
    n_total = x_all.shape[0]
    xf = x_all.reshape(n_total, C, HH * WW)
    xp = np.zeros((n_total, C, XWP), np.float32)
    xp[:, :, 0:HH * WW] = xf
    xp = _round_f32r(xp)

    n_cores = n_total // ns
    in_maps = []
    for i in range(n_cores):
        in_maps.append({
            "x": xp[i * ns:(i + 1) * ns],
            "warr": warr, "brd": brd, "wvg": wvg, "cc": cc, "g2b2": g2b2,
            "mjunk": mjunk,
        })
    return in_maps


def kernel(state_curr, state_targ, conv_w, conv_b, ln1_g, ln1_b,
           wq, bq, wk, bk, wv, bv, ln2_g, ln2_b):
    inputs = dict(state_curr=state_curr, state_targ=state_targ,
                  conv_w=conv_w, conv_b=conv_b, ln1_g=ln1_g, ln1_b=ln1_b,
                  wq=wq, bq=bq, wk=wk, bk=bk, wv=wv, bv=bv,
                  ln2_g=ln2_g, ln2_b=ln2_b)
    if "nc" not in _CACHE:
        _CACHE["nc"] = build_nc(NS)
    nc = _CACHE["nc"]
    in_maps = host_prep(inputs, NS)
    res = run_bass_kernel_spmd(nc, in_maps, list(range(N_CORES)), trace=False)
    outs = [res.results[i]["out"] for i in range(N_CORES)]
    full = np.concatenate(outs, 0)            # (256, 256)
    nb = state_curr.shape[0]
    return np.concatenate([full[:nb], full[nb:]], axis=-1).astype(np.float32)


# revision 18
# speedup vs baseline: 1.0230x; 1.0230x over previous
"""Trainium2 Bass kernel for nn_Binder_MiniGrid (topk_masking).

Algebraic reduction: with q == bq constant, attention scores collapse to a
single linear functional of the LN'd conv features, which folds into two
extra output columns of the conv matmul (s1 = f @ (wk^T bq * ln1_g), mu).
Only the conv itself (451 GFLOP total) runs dense; k/v projections are never
materialized. Top-4 masking + softmax happens on a (1, 1968) score row per
sample (Max8). Pooling runs as a scalar_tensor_tensor accumulation chain on
the vector engine plus 3 tiny partition-sum matmuls per sample (keeping the
PE conv stream uninterrupted), and the final wv projection + LN2 + ReLU run
once per core on (32, 256).

Data parallel over 8 NeuronCores: 32 of the 256 stacked (curr; targ) samples
per core. Conv runs in float32r (TF32-like, 12-bit mantissa) at full PE rate;
measured end-to-end error vs the fp32 reference ~1.9e-4; ~1.14 ms/core.
"""
import sys
sys.path.insert(0, "/opt/trn_rl_repo")
import numpy as np

import concourse.bacc as bacc
import concourse.tile as tile
from concourse import mybir
from concourse.bass_types import AP as BassAP
from concourse.bass_utils import run_bass_kernel_spmd

F32 = mybir.dt.float32
F32R = mybir.dt.float32r
AF = mybir.ActivationFunctionType
ALU = mybir.AluOpType

B, C, HH, WW = 128, 32, 48, 48
E, KF, TOPK, EPS = 256, 8, 4, 1e-5
OH = 41
L = OH * OH            # 1681
NS = 32                # samples per core
N_CORES = 8
NT = 16                # M-tiles per sample: 16 x 123 = 1968 flat 48-wide positions
XW = 2308              # X4 tile width (48*48 data + tap/slice slack)
XWP = XW + 3           # padded host row width (j-shift headroom)
NE = E + 2             # 258: conv channels + s1 + mu columns
NEG = -1.0e9

_CACHE = {}


def _round_f32r(a):
    """Round fp32 to fp32r (round-to-nearest into 12-bit mantissa)."""
    bits = np.ascontiguousarray(a, np.float32).view(np.uint32).astype(np.uint64)
    r = ((bits + 0x800) & ~np.uint64(0xFFF)).astype(np.uint32)
    return r.view(np.float32).reshape(a.shape)


def build_nc(ns=NS, pipelined=True, bufs=4, psum_bufs=None):
    nc = bacc.Bacc()
    x_t = nc.dram_tensor("x", [ns, C, XWP], F32R, kind="ExternalInput")
    warr_ap = nc.dram_tensor("warr", [128, 16 * NE], F32R, kind="ExternalInput").ap()
    brd_ap = nc.dram_tensor("brd", [128, NE], F32, kind="ExternalInput").ap()
    wvg_ap = nc.dram_tensor("wvg", [128, 2 * E], F32, kind="ExternalInput").ap()
    cc_ap = nc.dram_tensor("cc", [2, E], F32, kind="ExternalInput").ap()
    g2b2_ap = nc.dram_tensor("g2b2", [NS, 2 * E], F32, kind="ExternalInput").ap()
    mjunk_ap = nc.dram_tensor("mjunk", [1, 123 * NT], F32, kind="ExternalInput").ap()
    out_ap = nc.dram_tensor("out", [ns, E], F32, kind="ExternalOutput").ap()

    with tile.TileContext(nc) as tc:
        with tc.tile_pool(name="const", bufs=1) as cpool, \
             tc.tile_pool(name="samp", bufs=bufs) as sp, \
             tc.tile_pool(name="small", bufs=3) as smp, \
             tc.tile_pool(name="rows", bufs=2) as rp, \
             tc.tile_pool(name="psum", bufs=psum_bufs or bufs, space="PSUM") as pp, \
             tc.tile_pool(name="psum_acc", bufs=1, space="PSUM") as pacc:

            # ---- static loads
            w_sb = cpool.tile([128, 16 * NE], F32R)
            nc.sync.dma_start(w_sb[:], warr_ap[:])
            brd = cpool.tile([128, NE], F32)
            nc.sync.dma_start(brd[:], brd_ap[:])
            wvg = cpool.tile([128, 2 * E], F32)
            nc.sync.dma_start(wvg[:], wvg_ap[:])
            cc = cpool.tile([2, E], F32)
            nc.sync.dma_start(cc[:], cc_ap[:])
            g2b2 = cpool.tile([NS, 2 * E], F32)
            nc.sync.dma_start(g2b2[:], g2b2_ap[:])
            textra = cpool.tile([2, NS], F32)
            nc.vector.memset(textra[0:2, :], 1.0)  # row 0 overwritten by T DMA below
            c_eeps = cpool.tile([128, 1], F32)
            nc.vector.memset(c_eeps[:], float(E) * EPS)
            c_eps = cpool.tile([NS, 1], F32)
            nc.vector.memset(c_eps[:], EPS)
            ones_col = cpool.tile([128, 1], F32)
            nc.vector.memset(ones_col[:], 1.0)
            mjunk = cpool.tile([1, 123 * NT], F32)
            nc.sync.dma_start(mjunk[:], mjunk_ap[:])

            pooled_all = pacc.tile([128, 3 * ns], F32)
            nc.vector.memset(pooled_all[:], 0.0)

            prev = None
            for s in range(ns):
                # ---- X4: partition (c*4+j) = x[s, c, j:j+XW]  (single DMA)
                x4 = sp.tile([128, XW], F32R)
                src = BassAP(x_t, s * C * XWP, [[XWP, C], [1, 4], [1, XW]])
                nc.sync.dma_start(x4[:, :], src)

                f_s = sp.tile([128, NT * NE], F32)
                ss = smp.tile([128, NT], F32)   # sum of squares per position

                for t in range(NT):
                    pt = pp.tile([128, NE], F32)
                    for ki in range(16):
                        kh, jb = ki // 2, ki % 2
                        off = kh * 48 + jb * 4 + 123 * t
                        lhsT = x4[:, off:off + 123]
                        rhs = w_sb[:, ki * NE:(ki + 1) * NE]
                        nc.tensor.matmul(pt[:123, :], lhsT, rhs,
                                         start=(ki == 0), stop=(ki == 15),
                                         skip_group_check=True)
                    ft = f_s[:, t * NE:(t + 1) * NE]
                    nc.vector.tensor_add(ft[:123, :], pt[:123, :], brd[:123, :])
                    sq = smp.tile([128, E], F32, tag="sq")
                    nc.scalar.activation(sq[:123, :], ft[:123, 0:E], AF.Square,
                                         accum_out=ss[:123, t:t + 1])

                # ---- scores on (123, NT)
                fv = f_s[:, :].rearrange("p (t e) -> p t e", t=NT)
                mu_v = fv[:123, :, E + 1]
                s1_v = fv[:123, :, E]
                sc1 = smp.tile([128, NT], F32, tag="sc1")
                nc.vector.tensor_mul(sc1[:123, :], mu_v, mu_v)
                nc.vector.scalar_tensor_tensor(sc1[:123, :], sc1[:123, :],
                                               -float(E), ss[:123, :],
                                               op0=ALU.mult, op1=ALU.add)
                sig = smp.tile([128, NT], F32, tag="sig")
                nc.scalar.activation(sig[:123, :], sc1[:123, :], AF.Sqrt,
                                     bias=c_eeps[:123, 0:1])
                inv2 = smp.tile([128, NT], F32, tag="inv2")
                nc.vector.reciprocal(inv2[:123, :], sig[:123, :])
                # host folded "- S*mu" into the s1 column: score = s1_v * inv2
                scs = smp.tile([128, NT], F32, tag="scs")
                nc.vector.tensor_mul(scs[:123, :], s1_v, inv2[:123, :])

                # ---- row / top-k (row order: flat p = oh*48 + ow; ow>=41 junk)
                row = rp.tile([1, 123 * NT], F32, tag="row")
                nc.sync.dma_start(row[0:1, :], scs[:123, :])
                # additive -1e9 at wrap positions (ow >= 41), in (p*NT + t) order
                nc.vector.tensor_add(row[:], row[:], mjunk[:])
                m8 = smp.tile([1, 8], F32, tag="m8")
                nc.vector.max(out=m8[:], in_=row[:])
                nvmax = smp.tile([1, 1], F32, tag="nvmax")
                nc.vector.tensor_scalar(nvmax[:], m8[:, 0:1], -1.0, None,
                                        op0=ALU.mult)
                e4 = smp.tile([1, 4], F32, tag="e4")
                sumw = smp.tile([1, 1], F32, tag="sumw")
                nc.scalar.activation(e4[:], m8[:, 0:4], AF.Exp,
                                     bias=nvmax[0:1, 0:1], accum_out=sumw[:])
                lnw = smp.tile([1, 1], F32, tag="lnw")
                nc.scalar.activation(lnw[:], sumw[:], AF.Ln)
                nvl = smp.tile([1, 1], F32, tag="nvl")
                nc.vector.tensor_sub(nvl[:], nvmax[:], lnw[:])
                erow = rp.tile([1, 123 * NT], F32, tag="erow")
                # erow = exp(row - vmax - ln(sumw)) * (row >= kth)
                nc.scalar.activation(erow[:], row[:], AF.Exp,
                                     bias=nvl[0:1, 0:1])
                nc.vector.scalar_tensor_tensor(erow[:], row[:], m8[0:1, 3:4],
                                               erow[:], op0=ALU.is_ge,
                                               op1=ALU.mult)

                # ---- back to (123, NT), * inv2
                w_s = smp.tile([128, NT], F32, tag="w_s")
                nc.sync.dma_start(w_s[:123, :], erow[0:1, :])
                nc.vector.tensor_mul(w_s[:123, :], w_s[:123, :], inv2[:123, :])

                # ---- pooled: DVE weighted accumulation over tiles, then 3
                # tiny PE matmuls (vs ones) for the partition reduction.
                # Software-pipelined one sample back so the score->DMA->softmax
                # chain hides under the next conv.
                def emit_pooled(si, fsi, wsi, first, last):
                    acc = smp.tile([128, NE], F32, tag="acc")
                    nc.vector.tensor_scalar(acc[:123, :], fsi[:123, 0:NE],
                                            wsi[:123, 0:1], None, op0=ALU.mult)
                    for t in range(1, NT):
                        nc.vector.scalar_tensor_tensor(
                            acc[:123, :], fsi[:123, t * NE:(t + 1) * NE],
                            wsi[:123, t:t + 1], acc[:123, :],
                            op0=ALU.mult, op1=ALU.add)
                    for ci, (c0, cm) in enumerate(((0, 128), (128, 128), (256, 2))):
                        nc.tensor.matmul(
                            pooled_all[0:cm, 3 * si + ci:3 * si + ci + 1],
                            acc[:123, c0:c0 + cm], ones_col[:123, 0:1],
                            start=(first and ci == 0), stop=(last and ci == 2),
                            skip_group_check=True)
                if pipelined:
                    if prev is not None:
                        emit_pooled(prev[0], prev[1], prev[2], prev[0] == 0, False)
                    prev = (s, f_s, w_s)
                else:
                    emit_pooled(s, f_s, w_s, s == 0, s == ns - 1)

            if pipelined:
                emit_pooled(prev[0], prev[1], prev[2], prev[0] == 0, True)

            # ---- tail: final projection + LN2 + relu
            psb = cpool.tile([128, 3 * ns], F32)
            nc.scalar.copy(psb[:], pooled_all[:])
            nc.sync.dma_start(textra[0:1, 0:ns], psb[1:2, 2::3])
            outp = pacc.tile([NS, E], F32, tag="outp")
            nc.tensor.matmul(outp[0:ns, :], psb[:, 0::3][:, 0:ns], wvg[:, 0:E],
                             start=True, stop=False, skip_group_check=True)
            nc.tensor.matmul(outp[0:ns, :], psb[:, 1::3][:, 0:ns], wvg[:, E:2 * E],
                             start=False, stop=False, skip_group_check=True)
            nc.tensor.matmul(outp[0:ns, :], textra[:, 0:ns], cc[:],
                             start=False, stop=True, skip_group_check=True)

            srow = cpool.tile([NS, 1], F32)
            nc.vector.reduce_sum(srow[0:ns, :], outp[0:ns, :],
                                 axis=mybir.AxisListType.X)
            nmu = cpool.tile([NS, 1], F32)
            nc.vector.tensor_scalar(nmu[0:ns, :], srow[0:ns, :], -1.0 / E, None,
                                    op0=ALU.mult)
            cent = cpool.tile([NS, E], F32)
            nc.vector.tensor_scalar(cent[0:ns, :], outp[0:ns, :],
                                    nmu[0:ns, 0:1], None, op0=ALU.add)
            sq2 = cpool.tile([NS, E], F32)
            ssq = cpool.tile([NS, 1], F32)
            nc.scalar.activation(sq2[0:ns, :], cent[0:ns, :], AF.Square,
                                 accum_out=ssq[0:ns, :])
            sigf = cpool.tile([NS, 1], F32)
            nc.scalar.activation(sigf[0:ns, :], ssq[0:ns, :], AF.Sqrt,
                                 bias=c_eps[0:ns, 0:1], scale=1.0 / E)
            invf = cpool.tile([NS, 1], F32)
            nc.vector.reciprocal(invf[0:ns, :], sigf[0:ns, :])
            nc.vector.tensor_scalar(cent[0:ns, :], cent[0:ns, :],
                                    invf[0:ns, 0:1], None, op0=ALU.mult)
            nc.vector.tensor_mul(cent[0:ns, :], cent[0:ns, :], g2b2[0:ns, 0:E])
            nc.vector.tensor_add(cent[0:ns, :], cent[0:ns, :], g2b2[0:ns, E:2 * E])
            yout = cpool.tile([NS, E], F32)
            nc.scalar.activation(yout[0:ns, :], cent[0:ns, :], AF.Relu)
            nc.sync.dma_start(out_ap[:], yout[0:ns, :])

    nc.finalize()
    return nc


def host_prep(inputs, ns=NS):
    """Build per-core input maps + the stacked sample array."""
    conv_w = np.asarray(inputs["conv_w"], np.float32)
    conv_b = np.asarray(inputs["conv_b"], np.float32)
    g1 = np.asarray(inputs["ln1_g"], np.float32)
    b1 = np.asarray(inputs["ln1_b"], np.float32)
    wk = np.asarray(inputs["wk"], np.float32)
    bk = np.asarray(inputs["bk"], np.float32)
    bq = np.asarray(inputs["bq"], np.float32)
    wv = np.asarray(inputs["wv"], np.float32)
    bv = np.asarray(inputs["bv"], np.float32)
    g2 = np.asarray(inputs["ln2_g"], np.float32)
    b2 = np.asarray(inputs["ln2_b"], np.float32)

    W2 = conv_w.transpose(1, 2, 3, 0).reshape(C * KF * KF, E)  # [(c,kh,kw), e]
    u = wk.T @ bq
    ug = u * g1
    S = float(ug.sum())
    col_s1 = W2 @ ug - S * (W2 @ (np.ones(E, np.float32) / E))
    # fold "- S*mu" into the s1 column so the device skips that op:
    col_mu = W2 @ (np.ones(E, np.float32) / E)
    W_aug = np.concatenate([W2, col_s1[:, None], col_mu[:, None]], 1)
    b_mu = float(conv_b.mean())
    b_s1 = float(conv_b @ ug) - S * b_mu
    bias_aug = np.concatenate([conv_b, [b_s1], [b_mu]]).astype(np.float32)

    # rearrange W_aug into the 16 (kh, jb) chunks, k = c*4 + j
    warr = np.zeros((128, 16 * NE), np.float32)
    for ki in range(16):
        kh, jb = ki // 2, ki % 2
        for c in range(C):
            for j in range(4):
                warr[c * 4 + j, ki * NE:(ki + 1) * NE] = \
                    W_aug[c * 64 + kh * 8 + jb * 4 + j]
    warr = _round_f32r(warr)

    brd = np.tile(bias_aug[None, :], (128, 1)).astype(np.float32)

    sqE = np.sqrt(np.float32(E))
    wvg_m = (wv * g1[None, :]) * sqE          # (f, e)
    wvgT = wvg_m.T                            # (e, f)
    wvg = np.zeros((128, 2 * E), np.float32)
    wvg[:, 0:E] = wvgT[0:128, :]
    wvg[:, E:2 * E] = wvgT[128:256, :]

    vec_t = wvg_m.sum(axis=1)                 # sqE * sum_e wv*g1
    const_vec = wv @ b1 + bv
    cc = np.stack([-vec_t, const_vec]).astype(np.float32)

    g2b2 = np.zeros((NS, 2 * E), np.float32)
    g2b2[:, 0:E] = g2[None, :]
    g2b2[:, E:2 * E] = b2[None, :]

    mjunk = np.zeros((1, 123 * NT), np.float32)
    for p in range(123):
        for t in range(NT):
            if (123 * t + p) % 48 >= OH:
                mjunk[0, p * NT + t] = NEG

    x_all = np.concatenate([np.asarray(inputs["state_curr"], np.float32),
                            np.asarray(inputs["state_targ"], np.float32)], 0)
    n_total = x_all.shape[0]
    xf = x_all.reshape(n_total, C, HH * WW)
    xp = np.zeros((n_total, C, XWP), np.float32)
    xp[:, :, 0:HH * WW] = xf
    xp = _round_f32r(xp)

    n_cores = n_total // ns
    in_maps = []
    for i in range(n_cores):
        in_maps.append({
            "x": xp[i * ns:(i + 1) * ns],
            "warr": warr, "brd": brd, "wvg": wvg, "cc": cc, "g2b2": g2b2,
            "mjunk": mjunk,
        })
    return in_maps


def kernel(state_curr, state_targ, conv_w, conv_b, ln1_g, ln1_b,
           wq, bq, wk, bk, wv, bv, ln2_g, ln2_b):
    inputs = dict(state_curr=state_curr, state_targ=state_targ,
                  conv_w=conv_w, conv_b=conv_b, ln1_g=ln1_g, ln1_b=ln1_b,
                  wq=wq, bq=bq, wk=wk, bk=bk, wv=wv, bv=bv,
                  ln2_g=ln2_g, ln2_b=ln2_b)
    if "nc" not in _CACHE:
        _CACHE["nc"] = build_nc(NS)
    nc = _CACHE["nc"]
    in_maps = host_prep(inputs, NS)
    res = run_bass_kernel_spmd(nc, in_maps, list(range(N_CORES)), trace=False)
    outs = [res.results[i]["out"] for i in range(N_CORES)]
    full = np.concatenate(outs, 0)            # (256, 256)
    nb = state_curr.shape[0]
    return np.concatenate([full[:nb], full[nb:]], axis=-1).astype(np.float32)
